# revision 38
# baseline (speedup 1.0000x reference)
"""Bipartite (Bahdanau) attention layer on 8 Trainium2 NeuronCores.

reference:
    S = student_feats @ W_s + b_s                       # [N_s, H]
    C = college_feats @ W_c + b_c                       # [N_c, H]
    scores[s, c] = tanh(S[s] + C[c]) . w_a + b_a        # [N_s, N_c]
    weights = softmax(scores, axis=-1)
    attended = weights @ college_feats                  # [N_s, D]
    returns (attended, weights)

Sharding: students (N_s=2048) split 8 ways; every core holds the full
college set and params, no collectives.

Per-core kernel structure:
  - S_T = (student @ W_s)^T  [H=128 part, 256 students]   (PE matmul)
  - C_T = (college @ W_c + b_s + b_c)^T [H=128, 1024]     (PE matmul; both
    biases folded into C since only S+C is ever used; b_a cancels in
    softmax and weights/attended are the only outputs)
  - per student s: T_s = tanh(C_T + S_T[:, s]) via ONE ScalarE activation
    (bias = per-partition column of S_T). This is the bottleneck:
    256 instrs x (1024+222)cyc @1.2GHz ~= 266us.
  - scores row-block: per student, matmul with stationary mask M_s =
    w_a ⊗ e_s ([128h x 128m], column s = w_a) accumulating into a PSUM
    [128 students x 1024] block (128-matmul accumulation group).
  - softmax without max subtraction (|scores| <~ sum|w_a| ~ 6, exp is
    safe in fp32); exp + row-sum fused via activation accum_out.
  - attended: PE-transpose the weight block, then fp32 matmuls against
    college_feats.
"""

import os
from contextlib import ExitStack

import numpy as np

N_CORES = 8
N_S = 2048
N_C = 1024
D = 256  # INPUT_DIM
H = 128  # HIDDEN_DIM
P = 128  # partitions
NS_LOC = N_S // N_CORES  # 256 students per core
SBLK = NS_LOC // P  # 2 student blocks of 128

# Lane split per 128-student block. "A" students get tanh straight from
# ScalarE. "D"/"G" students go through the exp-product route:
# r = 1/(1 + e^{2S}e^{2C}) with tanh(z) = 1 - 2r folded into the softmax exp
# via per-partition scale/bias. For "D" the multiply-add AND the fast
# reciprocal run on VectorE; for "G" the multiply-add runs on GpSimd and only
# the reciprocal on VectorE. Counts balance ~1039ns/student (ScalarE tanh),
# ~663+1130ns (VectorE ts+recip), ~1130ns (recip only) + ~2030ns GpSimd ts.
LANE_COUNTS = (("A", 70), ("G", 58))


def _lanes():
    slots = []
    for kind, cnt in LANE_COUNTS:
        slots += [((i + 0.5) / cnt, kind) for i in range(cnt)]
    return [k for _, k in sorted(slots)]


LANES = _lanes()
assert len(LANES) == P

_cache = {}


def _build_nc():
    import concourse.tile as tile
    from concourse import bacc, mybir

    f32 = mybir.dt.float32
    f16 = mybir.dt.float16
    AF = mybir.ActivationFunctionType

    nc = bacc.Bacc("TRN2", target_bir_lowering=False, debug=False)

    studentT_d = nc.declare_dram_parameter("studentT", [D, NS_LOC], f32, isOutput=False)
    collegeT_d = nc.declare_dram_parameter("collegeT", [D, N_C], f32, isOutput=False)
    college_d = nc.declare_dram_parameter("college", [N_C, D], f32, isOutput=False)
    ws_d = nc.declare_dram_parameter("w_s", [D, H], f32, isOutput=False)
    wc_d = nc.declare_dram_parameter("w_c", [D, H], f32, isOutput=False)
    bsc_d = nc.declare_dram_parameter("b_sc", [1, H], f32, isOutput=False)
    masks_d = nc.declare_dram_parameter("masks", [P, P * P], f16, isOutput=False)
    ident_d = nc.declare_dram_parameter("ident", [P, P], f32, isOutput=False)
    scale_d = nc.declare_dram_parameter("scale_col", [P, 1], f32, isOutput=False)
    biasc_d = nc.declare_dram_parameter("bias_col", [P, 1], f32, isOutput=False)
    att_d = nc.declare_dram_parameter("attended", [NS_LOC, D], f32, isOutput=True)
    wout_d = nc.declare_dram_parameter("weights", [NS_LOC, N_C], f32, isOutput=True)

    with tile.TileContext(nc) as tc, ExitStack() as ctx:
        const = ctx.enter_context(tc.tile_pool(name="const", bufs=1))
        tpool = ctx.enter_context(tc.tile_pool(name="tpool", bufs=10))
        spool = ctx.enter_context(tc.tile_pool(name="spool", bufs=2))

        # ---- constant / input tiles -----------------------------------
        studentT = [
            const.tile([P, NS_LOC], f32, name=f"studentT{i}", tag=f"studentT{i}")
            for i in range(2)
        ]
        collegeT = [
            const.tile([P, N_C], f32, name=f"collegeT{i}", tag=f"collegeT{i}")
            for i in range(2)
        ]
        college = [
            const.tile([P, D], f32, name=f"college{j}", tag=f"college{j}")
            for j in range(8)
        ]
        college16 = [
            const.tile([P, D], f16, name=f"college16_{j}", tag=f"college16_{j}")
            for j in range(8)
        ]
        ws = [
            const.tile([P, H], f32, name=f"ws{i}", tag=f"ws{i}") for i in range(2)
        ]
        wc = [
            const.tile([P, H], f32, name=f"wc{i}", tag=f"wc{i}") for i in range(2)
        ]
        bsc = const.tile([1, H], f32, name="bsc", tag="bsc")
        ones = const.tile([1, 512], f32, name="ones", tag="ones")
        ident = const.tile([P, P], f32, name="ident", tag="ident")
        scol = const.tile([P, 1], f32, name="scol", tag="scol")
        bcol = const.tile([P, 1], f32, name="bcol", tag="bcol")
        masks = const.tile([P, P * P], f16, name="masks", tag="masks")
        s_t = const.tile([P, NS_LOC], f32, name="s_t", tag="s_t")
        c_t = const.tile([P, N_C], f32, name="c_t", tag="c_t")
        e_s = const.tile([P, NS_LOC], f32, name="e_s", tag="e_s")
        e_c = const.tile([P, N_C], f32, name="e_c", tag="e_c")

        # critical path first: C_T (then S_T) gate the tanh loop. collegeT
        # arrives in column halves so C_T chunk 0 can start on half the data.
        nc.sync.dma_start(bsc[:], bsc_d[:])
        for i in range(2):
            nc.sync.dma_start(wc[i][:], wc_d[i * P : (i + 1) * P, :])
        for cc in range(2):
            csl = slice(cc * 512, (cc + 1) * 512)
            for i in range(2):
                nc.sync.dma_start(
                    collegeT[i][:, csl], collegeT_d[i * P : (i + 1) * P, csl]
                )
        for i in range(2):
            nc.sync.dma_start(studentT[i][:], studentT_d[i * P : (i + 1) * P, :])
            nc.sync.dma_start(ws[i][:], ws_d[i * P : (i + 1) * P, :])
        nc.vector.memset(ones[:], 1.0)
        # masks (host-built w_a diag blocks), chunked so early students'
        # mask slices become available before the whole 4MB lands
        for mc in range(4):
            nc.sync.dma_start(
                masks[:, mc * 4096 : (mc + 1) * 4096],
                masks_d[:, mc * 4096 : (mc + 1) * 4096],
            )
        nc.sync.dma_start(scol[:], scale_d[:])
        nc.sync.dma_start(bcol[:], biasc_d[:])
        # attended inputs are not needed until much later
        for j in range(8):
            nc.sync.dma_start(college[j][:], college_d[j * P : (j + 1) * P, :])
            nc.gpsimd.tensor_copy(college16[j][:], college[j][:])
        nc.sync.dma_start(ident[:], ident_d[:])

        # ---- S_T and C_T ----------------------------------------------
        with tc.tile_pool(name="pinit", bufs=2, space="PSUM") as pinit:
            # Warm up the PE HAM clock gate (cold 1.2GHz -> 2.4GHz needs
            # ~3.4us of busy) while the input DMAs are still in flight.
            warm_rhs = const.tile([P, 512], f32, name="warm_rhs", tag="warm_rhs")
            warm_lhs = const.tile([P, P], f32, name="warm_lhs", tag="warm_lhs")
            nc.vector.memset(warm_rhs[:], 0.0)
            nc.vector.memset(warm_lhs[:], 0.0)
            warm_ps = pinit.tile([P, 512], f32, name="warm_ps", tag="warm_ps")
            for _ in range(4):
                nc.tensor.matmul(warm_ps[:], warm_lhs[:], warm_rhs[:], start=True, stop=True)

            for cc in range(2):
                ct_ps = pinit.tile([P, 512], f32, name=f"ct_ps{cc}", tag="ct_ps")
                sl = slice(cc * 512, (cc + 1) * 512)
                # (b_s + b_c)[h] broadcast along colleges first (rank-1
                # matmul whose inputs arrive earliest), then the weights.
                nc.tensor.matmul(ct_ps[:], bsc[:], ones[:], start=True, stop=False)
                nc.tensor.matmul(ct_ps[:], wc[0][:], collegeT[0][:, sl], start=False, stop=False)
                nc.tensor.matmul(ct_ps[:], wc[1][:], collegeT[1][:, sl], start=False, stop=True)
                nc.scalar.copy(c_t[:, sl], ct_ps[:])

            st_ps = pinit.tile([P, NS_LOC], f32, name="st_ps", tag="st_ps")
            nc.tensor.matmul(st_ps[:], ws[0][:], studentT[0][:], start=True, stop=False)
            nc.tensor.matmul(st_ps[:], ws[1][:], studentT[1][:], start=False, stop=True)
            nc.scalar.copy(s_t[:], st_ps[:])

        if any(k != "A" for k in LANES):
            # VectorE lane inputs: e^{2C} and e^{2S}
            nc.scalar.activation(e_c[:], c_t[:], AF.Exp, scale=2.0)
            nc.scalar.activation(e_s[:], s_t[:], AF.Exp, scale=2.0)

        # ---- main loop over student blocks ----------------------------
        with (
            tc.tile_pool(name="psc", bufs=2, space="PSUM") as psc,
            tc.tile_pool(name="pmisc", bufs=2, space="PSUM") as pmisc,
        ):
            from concourse.dve_ops import (
                RECIP_APPROX_FAST_CONSTS as RC,
                RECIPROCAL_APPROX_FAST,
            )

            for sb in range(SBLK):
                ps0 = psc.tile([P, 512], f32, name=f"ps0_{sb}", tag="ps0")
                ps1 = psc.tile([P, 512], f32, name=f"ps1_{sb}", tag="ps1")
                for s in range(P):
                    col = slice(sb * P + s, sb * P + s + 1)
                    lane = LANES[s]
                    if lane == "A":
                        # ScalarE lane: rhs = tanh(C + S_s) in fp16
                        t = tpool.tile([P, N_C], f16, name=f"t{sb}_{s}", tag="t", bufs=8)
                        nc.scalar.activation(t[:], c_t[:], AF.Tanh, bias=s_t[:, col])
                        rhs = t
                    else:
                        # exp-product lanes: rhs = 1/(1 + e^{2S_s} e^{2C})
                        vtag, vb = ("vd", 3) if lane == "D" else ("vg", 6)
                        v = tpool.tile([P, N_C], f32, name=f"v{sb}_{s}", tag=vtag, bufs=vb)
                        eng = nc.vector if lane == "D" else nc.gpsimd
                        eng.tensor_scalar(
                            v[:], e_c[:], e_s[:, col], 1.0,
                            op0=mybir.AluOpType.mult, op1=mybir.AluOpType.add,
                        )
                        r = tpool.tile([P, N_C], f16, name=f"r{sb}_{s}", tag="r", bufs=8)
                        nc.vector._custom_dve(
                            RECIPROCAL_APPROX_FAST, out=r[:], in0=v[:],
                            s0=RC["s0"], s1=RC["s1"], imm2=RC["imm2"],
                        )
                        rhs = r
                    m = masks[:, s * P : (s + 1) * P]
                    nc.tensor.matmul(
                        ps0[:], m, rhs[:, 0:512], start=(s == 0), stop=(s == P - 1)
                    )
                    nc.tensor.matmul(
                        ps1[:], m, rhs[:, 512:1024], start=(s == 0), stop=(s == P - 1)
                    )

                # softmax over colleges (no max subtraction needed; |scores|<~6).
                # Per-partition scale/bias map each lane's accumulator to
                # exp(scores): ACT rows exp(ps), exp-lane rows exp(Wsum - 2 ps).
                # Transposes of each exp half start while the other half's exp
                # is still running; normalization happens at the very end.
                e01 = [
                    spool.tile([P, 512], f32, name=f"e{h}_{sb}", tag=f"e{h}")
                    for h in range(2)
                ]
                rs01 = [
                    spool.tile([P, 1], f32, name=f"rs{h}_{sb}", tag=f"rs{h}")
                    for h in range(2)
                ]
                wt = []
                for h, ps in enumerate((ps0, ps1)):
                    nc.scalar.activation(
                        e01[h][:], ps[:], AF.Exp,
                        scale=scol[:], bias=bcol[:], accum_out=rs01[h][:],
                    )
                    # f16 copy feeds the DMA-xbar transposes (2-byte only) and
                    # the f16 attended matmul; weights output stays fp32.
                    e16 = spool.tile([P, 512], f16, name=f"e16_{h}_{sb}", tag=f"e16_{h}")
                    nc.vector.tensor_copy(e16[:], e01[h][:])
                    for q in range(4):
                        cb = h * 4 + q
                        wtile = spool.tile([P, P], f16, name=f"wt{sb}_{cb}", tag=f"wt{cb}")
                        nc.sync.dma_start_transpose(wtile[:], e16[:, q * P : (q + 1) * P])
                        wt.append(wtile)
                rsum = spool.tile([P, 1], f32, name=f"rsum_{sb}", tag="rsum")
                nc.vector.tensor_add(rsum[:], rs01[0][:], rs01[1][:])
                rcp = spool.tile([P, 1], f32, name=f"rcp_{sb}", tag="rcp")
                nc.vector.reciprocal(rcp[:], rsum[:])
                attps = pmisc.tile([P, D], f32, name=f"attps{sb}", tag="attps")
                for cb in range(8):
                    nc.tensor.matmul(
                        attps[:], wt[cb][:], college16[cb][:],
                        start=(cb == 0), stop=(cb == 7),
                    )
                srow = slice(sb * P, (sb + 1) * P)
                w0 = spool.tile([P, 512], f32, name=f"w0_{sb}", tag="w0")
                w1 = spool.tile([P, 512], f32, name=f"w1_{sb}", tag="w1")
                nc.vector.tensor_scalar_mul(w0[:], e01[0][:], rcp[:])
                nc.sync.dma_start(wout_d[srow, 0:512], w0[:])
                nc.vector.tensor_scalar_mul(w1[:], e01[1][:], rcp[:])
                nc.sync.dma_start(wout_d[srow, 512:1024], w1[:])
                att = spool.tile([P, D], f32, name=f"att_{sb}", tag="att")
                nc.vector.tensor_scalar_mul(att[:], attps[:], rcp[:])
                nc.sync.dma_start(att_d[srow, :], att[:])

    nc.compile()
    return nc


def _get_nc():
    if "nc" not in _cache:
        _cache["nc"] = _build_nc()
    return _cache["nc"]


def _make_in_maps(student_feats, college_feats, W_s, b_s, W_c, b_c, w_a):
    f = np.float32
    # Per-partition softmax-exp transform: ScalarE-lane rows ([0, NA)) hold
    # scores directly; VectorE-lane rows hold sum_h w_a*r, and
    # scores = Wsum - 2 * that.
    wsum = float(np.sum(np.asarray(w_a, dtype=np.float64)))
    scale_col = np.ones((P, 1), dtype=f)
    bias_col = np.zeros((P, 1), dtype=f)
    for s in range(P):
        if LANES[s] != "A":
            scale_col[s] = -2.0
            bias_col[s] = wsum
    masks = np.zeros((P, P * P), dtype=np.float16)
    wa16 = np.asarray(w_a, dtype=np.float16)
    for s in range(P):
        masks[:, s * P + s] = wa16
    base = {
        "collegeT": np.ascontiguousarray(college_feats.T, dtype=f),
        "college": np.ascontiguousarray(college_feats, dtype=f),
        "w_s": np.ascontiguousarray(W_s, dtype=f),
        "w_c": np.ascontiguousarray(W_c, dtype=f),
        "b_sc": np.ascontiguousarray((b_s + b_c).reshape(1, H), dtype=f),
        "masks": masks,
        "ident": np.eye(P, dtype=f),
        "scale_col": scale_col,
        "bias_col": bias_col,
    }
    studentT = np.ascontiguousarray(student_feats.T, dtype=f)  # [D, N_S]
    return [
        dict(base, studentT=np.ascontiguousarray(studentT[:, c * NS_LOC : (c + 1) * NS_LOC]))
        for c in range(N_CORES)
    ]


def kernel(student_feats, college_feats, W_s, b_s, W_c, b_c, w_a, b_a):
    # b_a shifts every score equally, so it cancels in the softmax; neither
    # output (attended, weights) depends on it.
    del b_a
    student_feats = np.asarray(student_feats, dtype=np.float32)
    college_feats = np.asarray(college_feats, dtype=np.float32)
    W_s = np.asarray(W_s, dtype=np.float32)
    W_c = np.asarray(W_c, dtype=np.float32)
    b_s = np.asarray(b_s, dtype=np.float32)
    b_c = np.asarray(b_c, dtype=np.float32)
    w_a = np.asarray(w_a, dtype=np.float32)

    from concourse.bass_utils import run_bass_kernel_spmd

    nc = _get_nc()
    in_maps = _make_in_maps(student_feats, college_feats, W_s, b_s, W_c, b_c, w_a)
    res = run_bass_kernel_spmd(nc, in_maps, list(range(N_CORES)))
    attended = np.concatenate(
        [np.asarray(res.results[i]["attended"]) for i in range(N_CORES)], axis=0
    )
    weights = np.concatenate(
        [np.asarray(res.results[i]["weights"]) for i in range(N_CORES)], axis=0
    )
    return attended, weights


# revision 40
# speedup vs baseline: 1.1072x; 1.1072x over previous
"""Bipartite (Bahdanau) attention layer on 8 Trainium2 NeuronCores.

reference:
    S = student_feats @ W_s + b_s                       # [N_s, H]
    C = college_feats @ W_c + b_c                       # [N_c, H]
    scores[s, c] = tanh(S[s] + C[c]) . w_a + b_a        # [N_s, N_c]
    weights = softmax(scores, axis=-1)
    attended = weights @ college_feats                  # [N_s, D]
    returns (attended, weights)

Sharding: students (N_s=2048) split 8 ways; every core holds the full
college set and params, no collectives.

Per-core kernel structure:
  - S_T = (student @ W_s)^T  [H=128 part, 256 students]   (PE matmul)
  - C_T = (college @ W_c + b_s + b_c)^T [H=128, 1024]     (PE matmul; both
    biases folded into C since only S+C is ever used; b_a cancels in
    softmax and weights/attended are the only outputs)
  - per student s: T_s = tanh(C_T + S_T[:, s]) via ONE ScalarE activation
    (bias = per-partition column of S_T). This is the bottleneck:
    256 instrs x (1024+222)cyc @1.2GHz ~= 266us.
  - scores row-block: per student, matmul with stationary mask M_s =
    w_a ⊗ e_s ([128h x 128m], column s = w_a) accumulating into a PSUM
    [128 students x 1024] block (128-matmul accumulation group).
  - softmax without max subtraction (|scores| <~ sum|w_a| ~ 6, exp is
    safe in fp32); exp + row-sum fused via activation accum_out.
  - attended: PE-transpose the weight block, then fp32 matmuls against
    college_feats.
"""

import os
from contextlib import ExitStack

import numpy as np

N_CORES = 8
N_S = 2048
N_C = 1024
D = 256  # INPUT_DIM
H = 128  # HIDDEN_DIM
P = 128  # partitions
NS_LOC = N_S // N_CORES  # 256 students per core
SBLK = NS_LOC // P  # 2 student blocks of 128

# Lane split per 128-student block. "A" students get tanh straight from
# ScalarE. "D"/"G" students go through the exp-product route:
# r = 1/(1 + e^{2S}e^{2C}) with tanh(z) = 1 - 2r folded into the softmax exp
# via per-partition scale/bias. For "D" the multiply-add AND the fast
# reciprocal run on VectorE; for "G" the multiply-add runs on GpSimd and only
# the reciprocal on VectorE. Counts balance ~1039ns/student (ScalarE tanh),
# ~663+1130ns (VectorE ts+recip), ~1130ns (recip only) + ~2030ns GpSimd ts.
LANE_COUNTS = (("A", 68), ("G", 60))


def _lanes():
    slots = []
    for kind, cnt in LANE_COUNTS:
        slots += [((i + 0.5) / cnt, kind) for i in range(cnt)]
    return [k for _, k in sorted(slots)]


LANES = _lanes()
assert len(LANES) == P

_cache = {}


def _build_nc():
    import concourse.tile as tile
    from concourse import bacc, mybir

    f32 = mybir.dt.float32
    f16 = mybir.dt.float16
    AF = mybir.ActivationFunctionType

    nc = bacc.Bacc("TRN2", target_bir_lowering=False, debug=False)

    studentT_d = nc.declare_dram_parameter("studentT", [D, NS_LOC], f32, isOutput=False)
    collegeT_d = nc.declare_dram_parameter("collegeT", [D, N_C], f32, isOutput=False)
    college_d = nc.declare_dram_parameter("college", [N_C, D], f32, isOutput=False)
    ws_d = nc.declare_dram_parameter("w_s", [D, H], f32, isOutput=False)
    wc_d = nc.declare_dram_parameter("w_c", [D, H], f32, isOutput=False)
    bsc_d = nc.declare_dram_parameter("b_sc", [1, H], f32, isOutput=False)
    masks_d = nc.declare_dram_parameter("masks", [P, P * P], f16, isOutput=False)
    ident_d = nc.declare_dram_parameter("ident", [P, P], f32, isOutput=False)
    scale_d = nc.declare_dram_parameter("scale_col", [P, 1], f32, isOutput=False)
    biasc_d = nc.declare_dram_parameter("bias_col", [P, 1], f32, isOutput=False)
    att_d = nc.declare_dram_parameter("attended", [NS_LOC, D], f32, isOutput=True)
    wout_d = nc.declare_dram_parameter("weights", [NS_LOC, N_C], f32, isOutput=True)

    with tile.TileContext(nc) as tc, ExitStack() as ctx:
        const = ctx.enter_context(tc.tile_pool(name="const", bufs=1))
        tpool = ctx.enter_context(tc.tile_pool(name="tpool", bufs=10))
        spool = ctx.enter_context(tc.tile_pool(name="spool", bufs=2))

        # ---- constant / input tiles -----------------------------------
        studentT = [
            const.tile([P, NS_LOC], f32, name=f"studentT{i}", tag=f"studentT{i}")
            for i in range(2)
        ]
        collegeT = [
            const.tile([P, N_C], f32, name=f"collegeT{i}", tag=f"collegeT{i}")
            for i in range(2)
        ]
        college = [
            const.tile([P, D], f32, name=f"college{j}", tag=f"college{j}")
            for j in range(8)
        ]
        college16 = [
            const.tile([P, D], f16, name=f"college16_{j}", tag=f"college16_{j}")
            for j in range(8)
        ]
        ws = [
            const.tile([P, H], f32, name=f"ws{i}", tag=f"ws{i}") for i in range(2)
        ]
        wc = [
            const.tile([P, H], f32, name=f"wc{i}", tag=f"wc{i}") for i in range(2)
        ]
        bsc = const.tile([1, H], f32, name="bsc", tag="bsc")
        ones = const.tile([1, 512], f32, name="ones", tag="ones")
        ident = const.tile([P, P], f32, name="ident", tag="ident")
        scol = const.tile([P, 1], f32, name="scol", tag="scol")
        bcol = const.tile([P, 1], f32, name="bcol", tag="bcol")
        masks = const.tile([P, P * P], f16, name="masks", tag="masks")
        s_t = const.tile([P, NS_LOC], f32, name="s_t", tag="s_t")
        c_t = const.tile([P, N_C], f32, name="c_t", tag="c_t")
        e_s = const.tile([P, NS_LOC], f32, name="e_s", tag="e_s")
        e_c = const.tile([P, N_C], f32, name="e_c", tag="e_c")

        # critical path first: C_T (then S_T) gate the tanh loop. collegeT
        # arrives in column halves so C_T chunk 0 can start on half the data.
        nc.sync.dma_start(bsc[:], bsc_d[:])
        for i in range(2):
            nc.sync.dma_start(wc[i][:], wc_d[i * P : (i + 1) * P, :])
        for cc in range(2):
            csl = slice(cc * 512, (cc + 1) * 512)
            for i in range(2):
                nc.sync.dma_start(
                    collegeT[i][:, csl], collegeT_d[i * P : (i + 1) * P, csl]
                )
        for i in range(2):
            nc.sync.dma_start(studentT[i][:], studentT_d[i * P : (i + 1) * P, :])
            nc.sync.dma_start(ws[i][:], ws_d[i * P : (i + 1) * P, :])
        nc.vector.memset(ones[:], 1.0)
        # masks (host-built w_a diag blocks), chunked so early students'
        # mask slices become available before the whole 4MB lands
        for mc in range(4):
            nc.sync.dma_start(
                masks[:, mc * 4096 : (mc + 1) * 4096],
                masks_d[:, mc * 4096 : (mc + 1) * 4096],
            )
        nc.sync.dma_start(scol[:], scale_d[:])
        nc.sync.dma_start(bcol[:], biasc_d[:])
        # attended inputs are not needed until much later
        for j in range(8):
            nc.sync.dma_start(college[j][:], college_d[j * P : (j + 1) * P, :])
            nc.gpsimd.tensor_copy(college16[j][:], college[j][:])
        nc.sync.dma_start(ident[:], ident_d[:])

        # ---- S_T and C_T ----------------------------------------------
        with tc.tile_pool(name="pinit", bufs=2, space="PSUM") as pinit:
            # Warm up the PE HAM clock gate (cold 1.2GHz -> 2.4GHz needs
            # ~3.4us of busy) while the input DMAs are still in flight.
            warm_rhs = const.tile([P, 512], f32, name="warm_rhs", tag="warm_rhs")
            warm_lhs = const.tile([P, P], f32, name="warm_lhs", tag="warm_lhs")
            nc.vector.memset(warm_rhs[:], 0.0)
            nc.vector.memset(warm_lhs[:], 0.0)
            warm_ps = pinit.tile([P, 512], f32, name="warm_ps", tag="warm_ps")
            for _ in range(4):
                nc.tensor.matmul(warm_ps[:], warm_lhs[:], warm_rhs[:], start=True, stop=True)

            for cc in range(2):
                ct_ps = pinit.tile([P, 512], f32, name=f"ct_ps{cc}", tag="ct_ps")
                sl = slice(cc * 512, (cc + 1) * 512)
                # (b_s + b_c)[h] broadcast along colleges first (rank-1
                # matmul whose inputs arrive earliest), then the weights.
                nc.tensor.matmul(ct_ps[:], bsc[:], ones[:], start=True, stop=False)
                nc.tensor.matmul(ct_ps[:], wc[0][:], collegeT[0][:, sl], start=False, stop=False)
                nc.tensor.matmul(ct_ps[:], wc[1][:], collegeT[1][:, sl], start=False, stop=True)
                nc.scalar.copy(c_t[:, sl], ct_ps[:])

            st_ps = pinit.tile([P, NS_LOC], f32, name="st_ps", tag="st_ps")
            nc.tensor.matmul(st_ps[:], ws[0][:], studentT[0][:], start=True, stop=False)
            nc.tensor.matmul(st_ps[:], ws[1][:], studentT[1][:], start=False, stop=True)
            nc.scalar.copy(s_t[:], st_ps[:])

        if any(k != "A" for k in LANES):
            # VectorE lane inputs: e^{2C} and e^{2S}
            nc.scalar.activation(e_c[:], c_t[:], AF.Exp, scale=2.0)
            nc.scalar.activation(e_s[:], s_t[:], AF.Exp, scale=2.0)

        # ---- main loop over student blocks ----------------------------
        with (
            tc.tile_pool(name="psc", bufs=2, space="PSUM") as psc,
            tc.tile_pool(name="pmisc", bufs=2, space="PSUM") as pmisc,
        ):
            from concourse.dve_ops import (
                RECIP_APPROX_FAST_CONSTS as RC,
                RECIPROCAL_APPROX_FAST,
            )

            for sb in range(SBLK):
                ps0 = psc.tile([P, 512], f32, name=f"ps0_{sb}", tag="ps0")
                ps1 = psc.tile([P, 512], f32, name=f"ps1_{sb}", tag="ps1")
                for s in range(P):
                    col = slice(sb * P + s, sb * P + s + 1)
                    lane = LANES[s]
                    if lane == "A":
                        # ScalarE lane: rhs = tanh(C + S_s) in fp16
                        t = tpool.tile([P, N_C], f16, name=f"t{sb}_{s}", tag="t", bufs=8)
                        nc.scalar.activation(t[:], c_t[:], AF.Tanh, bias=s_t[:, col])
                        rhs = t
                    else:
                        # exp-product lanes: rhs = 1/(1 + e^{2S_s} e^{2C})
                        vtag, vb = ("vd", 3) if lane == "D" else ("vg", 6)
                        v = tpool.tile([P, N_C], f32, name=f"v{sb}_{s}", tag=vtag, bufs=vb)
                        eng = nc.vector if lane == "D" else nc.gpsimd
                        eng.tensor_scalar(
                            v[:], e_c[:], e_s[:, col], 1.0,
                            op0=mybir.AluOpType.mult, op1=mybir.AluOpType.add,
                        )
                        r = tpool.tile([P, N_C], f16, name=f"r{sb}_{s}", tag="r", bufs=8)
                        nc.vector._custom_dve(
                            RECIPROCAL_APPROX_FAST, out=r[:], in0=v[:],
                            s0=RC["s0"], s1=RC["s1"], imm2=RC["imm2"],
                        )
                        rhs = r
                    m = masks[:, s * P : (s + 1) * P]
                    nc.tensor.matmul(
                        ps0[:], m, rhs[:, 0:512], start=(s == 0), stop=(s == P - 1)
                    )
                    nc.tensor.matmul(
                        ps1[:], m, rhs[:, 512:1024], start=(s == 0), stop=(s == P - 1)
                    )

                # softmax over colleges (no max subtraction needed; |scores|<~6).
                # Per-partition scale/bias map each lane's accumulator to
                # exp(scores): ACT rows exp(ps), exp-lane rows exp(Wsum - 2 ps).
                # Transposes of each exp half start while the other half's exp
                # is still running; normalization happens at the very end.
                e01 = [
                    spool.tile([P, 512], f32, name=f"e{h}_{sb}", tag=f"e{h}")
                    for h in range(2)
                ]
                rs01 = [
                    spool.tile([P, 1], f32, name=f"rs{h}_{sb}", tag=f"rs{h}")
                    for h in range(2)
                ]
                wt = []
                for h, ps in enumerate((ps0, ps1)):
                    nc.scalar.activation(
                        e01[h][:], ps[:], AF.Exp,
                        scale=scol[:], bias=bcol[:], accum_out=rs01[h][:],
                    )
                    for q in range(4):
                        cb = h * 4 + q
                        tp = pmisc.tile([P, P], f32, name=f"tp{sb}_{cb}", tag="tp")
                        nc.tensor.transpose(tp[:], e01[h][:, q * P : (q + 1) * P], ident[:])
                        # the PSUM->SBUF copy casts to f16 for the attended matmul
                        wtile = spool.tile([P, P], f16, name=f"wt{sb}_{cb}", tag=f"wt{cb}")
                        nc.vector.tensor_copy(wtile[:], tp[:])
                        wt.append(wtile)
                rsum = spool.tile([P, 1], f32, name=f"rsum_{sb}", tag="rsum")
                nc.vector.tensor_add(rsum[:], rs01[0][:], rs01[1][:])
                rcp = spool.tile([P, 1], f32, name=f"rcp_{sb}", tag="rcp")
                nc.vector.reciprocal(rcp[:], rsum[:])
                attps = pmisc.tile([P, D], f32, name=f"attps{sb}", tag="attps")
                for cb in range(8):
                    nc.tensor.matmul(
                        attps[:], wt[cb][:], college16[cb][:],
                        start=(cb == 0), stop=(cb == 7),
                    )
                srow = slice(sb * P, (sb + 1) * P)
                w0 = spool.tile([P, 512], f32, name=f"w0_{sb}", tag="w0")
                w1 = spool.tile([P, 512], f32, name=f"w1_{sb}", tag="w1")
                nc.vector.tensor_scalar_mul(w0[:], e01[0][:], rcp[:])
                nc.sync.dma_start(wout_d[srow, 0:512], w0[:])
                nc.vector.tensor_scalar_mul(w1[:], e01[1][:], rcp[:])
                nc.sync.dma_start(wout_d[srow, 512:1024], w1[:])
                att = spool.tile([P, D], f32, name=f"att_{sb}", tag="att")
                nc.vector.tensor_scalar_mul(att[:], attps[:], rcp[:])
                nc.sync.dma_start(att_d[srow, :], att[:])

    nc.compile()
    return nc


def _get_nc():
    if "nc" not in _cache:
        _cache["nc"] = _build_nc()
    return _cache["nc"]


def _make_in_maps(student_feats, college_feats, W_s, b_s, W_c, b_c, w_a):
    f = np.float32
    # Per-partition softmax-exp transform: ScalarE-lane rows ([0, NA)) hold
    # scores directly; VectorE-lane rows hold sum_h w_a*r, and
    # scores = Wsum - 2 * that.
    wsum = float(np.sum(np.asarray(w_a, dtype=np.float64)))
    scale_col = np.ones((P, 1), dtype=f)
    bias_col = np.zeros((P, 1), dtype=f)
    for s in range(P):
        if LANES[s] != "A":
            scale_col[s] = -2.0
            bias_col[s] = wsum
    masks = np.zeros((P, P * P), dtype=np.float16)
    wa16 = np.asarray(w_a, dtype=np.float16)
    for s in range(P):
        masks[:, s * P + s] = wa16
    base = {
        "collegeT": np.ascontiguousarray(college_feats.T, dtype=f),
        "college": np.ascontiguousarray(college_feats, dtype=f),
        "w_s": np.ascontiguousarray(W_s, dtype=f),
        "w_c": np.ascontiguousarray(W_c, dtype=f),
        "b_sc": np.ascontiguousarray((b_s + b_c).reshape(1, H), dtype=f),
        "masks": masks,
        "ident": np.eye(P, dtype=f),
        "scale_col": scale_col,
        "bias_col": bias_col,
    }
    studentT = np.ascontiguousarray(student_feats.T, dtype=f)  # [D, N_S]
    return [
        dict(base, studentT=np.ascontiguousarray(studentT[:, c * NS_LOC : (c + 1) * NS_LOC]))
        for c in range(N_CORES)
    ]


def kernel(student_feats, college_feats, W_s, b_s, W_c, b_c, w_a, b_a):
    # b_a shifts every score equally, so it cancels in the softmax; neither
    # output (attended, weights) depends on it.
    del b_a
    student_feats = np.asarray(student_feats, dtype=np.float32)
    college_feats = np.asarray(college_feats, dtype=np.float32)
    W_s = np.asarray(W_s, dtype=np.float32)
    W_c = np.asarray(W_c, dtype=np.float32)
    b_s = np.asarray(b_s, dtype=np.float32)
    b_c = np.asarray(b_c, dtype=np.float32)
    w_a = np.asarray(w_a, dtype=np.float32)

    from concourse.bass_utils import run_bass_kernel_spmd

    nc = _get_nc()
    in_maps = _make_in_maps(student_feats, college_feats, W_s, b_s, W_c, b_c, w_a)
    res = run_bass_kernel_spmd(nc, in_maps, list(range(N_CORES)))
    attended = np.concatenate(
        [np.asarray(res.results[i]["attended"]) for i in range(N_CORES)], axis=0
    )
    weights = np.concatenate(
        [np.asarray(res.results[i]["weights"]) for i in range(N_CORES)], axis=0
    )
    return attended, weights
